# revision 1
# baseline (speedup 1.0000x reference)
"""Causal multi-head attention block (B=2, T=2048, C=1024, H=16) on 8 TRN2
NeuronCores.

Sharding: Megatron-style tensor parallel over heads. Core r owns heads
{2r, 2r+1} = output dims [128r, 128r+128) of Wq/Wk/Wv, and the matching
128 input rows of Wo are NOT sharded that way; instead each core keeps the
full contraction over C for a 128-column shard of the final output:
out[:, 128r:128r+128] needs all of y, so cores AllGather their local yT
shards (concat on the partition axis gives exactly yT_full [C, B*T]) and
then compute outT_shard = Wo[128r:128r+128, :] @ yT_full locally.

Everything on-device is computed in the "transposed" orientation
(feature-major, token-minor) so the TensorEngine contraction axis always
sits on SBUF partitions and softmax normalization sums arrive for free via
a ones-column appended to V:

  qT/kT/vT [128, 4096] = W_shard @ x^T         (x^T passed from host)
  ST tile [128k, 512q] = kT_slice.T @ qT_slice  (contract d=64)
  PT = exp(ST * 1/sqrt(d))                      (no max-subtraction: logits
                                                 are ~N(0,1), |S|max ~ 6)
  causal mask: zero PT where k > q via gpsimd.affine_select
  yT [65, 512] += [v | 1].T @ PT                (row 64 = softmax denom)
  yT_norm = yT[0:64] * (1/yT[64]) broadcast across partitions

Inputs are bf16 (host-side cast); accumulation is f32 in PSUM.
"""

import numpy as np
import ml_dtypes

import concourse.bacc as bacc
import concourse.mybir as mybir
import concourse.tile as tile
from concourse.bass_utils import run_bass_kernel_spmd
from concourse.masks import make_identity

N_CORES = 8
B, T, C, H = 2, 2048, 1024, 16
D = 64                # head dim
HL = H // N_CORES     # heads per core = 2
DL = HL * D           # local feature dim = 128
TT = B * T            # 4096 tokens total
P = 128
NCH = C // P          # 8 contraction chunks
QCH = 512             # q-chunk (moving free dim)
NQC = T // QCH        # 4 q-chunks per batch entry
NKT = T // P          # 16 k-tiles per batch entry
SCALE = 1.0 / np.sqrt(D)

BF = mybir.dt.bfloat16
F32 = mybir.dt.float32
AF = mybir.ActivationFunctionType


def _scopy(nc, out, in_):
    nc.scalar.activation(out, in_, AF.Copy)


def build_graph():
    nc = bacc.Bacc("TRN2", target_bir_lowering=False, debug=False)

    xT = nc.dram_tensor("xT", [C, TT], BF, kind="ExternalInput")
    wqT = nc.dram_tensor("wqT", [C, DL], BF, kind="ExternalInput")
    wkT = nc.dram_tensor("wkT", [C, DL], BF, kind="ExternalInput")
    wvT = nc.dram_tensor("wvT", [C, DL], BF, kind="ExternalInput")
    woT = nc.dram_tensor("woT", [C, DL], BF, kind="ExternalInput")
    out = nc.dram_tensor("out", [DL, TT], F32, kind="ExternalOutput")

    with tile.TileContext(nc) as tc:
        with (
            tc.tile_pool(name="sb", bufs=1) as sb,
            tc.tile_pool(name="ps", bufs=1, space="PSUM") as ps,
            tc.tile_pool(name="dram", bufs=1, space="DRAM") as dram,
        ):
            # ---- phase 0: loads ----
            wq_sb = sb.tile([P, NCH, DL], BF, name="wq_sb")
            wk_sb = sb.tile([P, NCH, DL], BF, name="wk_sb")
            wv_sb = sb.tile([P, NCH, DL], BF, name="wv_sb")
            wo_sb = sb.tile([P, NCH, DL], BF, name="wo_sb")
            for wsb, wdram in (
                (wq_sb, wqT), (wk_sb, wkT), (wv_sb, wvT), (wo_sb, woT)
            ):
                nc.sync.dma_start(
                    wsb[:], wdram[:].rearrange("(a p) m -> p a m", p=P)
                )

            ident = sb.tile([P, P], BF, name="ident")
            make_identity(nc, ident)

            qT_sb = sb.tile([P, TT], BF, name="qT_sb")
            kT_sb = sb.tile([P, TT], BF, name="kT_sb")
            vT_sb = sb.tile([P, TT], BF, name="vT_sb")
            # v in natural layout, packed per 128-token tile as
            # [headA(64) | 1 | headB(64) | 1] -> 130 columns
            v_sb = sb.tile([P, TT // P, 2 * (D + 1)], BF, name="v_sb")
            nc.gpsimd.memset(v_sb[:], 1.0)

            yT_all = sb.tile([P, TT], BF, name="yT_all")

            with tc.tile_pool(name="xp", bufs=1) as xp:
                xT_sb = xp.tile([P, NCH, TT], BF, name="xT_sb")
                for ci in range(NCH):
                    for hf in range(2):
                        nc.sync.dma_start(
                            xT_sb[:, ci, hf * (TT // 2):(hf + 1) * (TT // 2)],
                            xT[ci * P:(ci + 1) * P,
                               hf * (TT // 2):(hf + 1) * (TT // 2)],
                        )

                # ---- phase 1: QKV projections (transposed layout) ----
                for tch in range(TT // QCH):
                    tsl = slice(tch * QCH, (tch + 1) * QCH)
                    for wsb, dst in (
                        (wq_sb, qT_sb), (wk_sb, kT_sb), (wv_sb, vT_sb)
                    ):
                        pj = ps.tile([P, QCH], F32, tag="mm", bufs=3)
                        for ci in range(NCH):
                            nc.tensor.matmul(
                                pj[:],
                                wsb[:, ci, :],
                                xT_sb[:, ci, tsl],
                                start=(ci == 0),
                                stop=(ci == NCH - 1),
                            )
                        _scopy(nc, dst[:, tsl], pj[:])
                    # ---- phase 1b: transpose this chunk of vT into v_sb ----
                    for t32 in range(tch * (QCH // P), (tch + 1) * (QCH // P)):
                        tr = ps.tile([P, P], BF, tag="mm", bufs=3)
                        nc.tensor.transpose(
                            tr[:], vT_sb[:, t32 * P:(t32 + 1) * P], ident[:]
                        )
                        out_ap = v_sb[:, t32, :].rearrange(
                            "p (h x) -> p h x", h=HL
                        )[:, :, 0:D]
                        in_ap = tr[:].rearrange("p (h x) -> p h x", h=HL)
                        _scopy(nc, out_ap, in_ap)

                # ---- phase 2: attention per (b, h) ----
                for b in range(B):
                    for h in range(HL):
                        rsl = slice(h * D, (h + 1) * D)
                        for jq in range(NQC):
                            q0 = b * T + jq * QCH
                            yt = ps.tile([D + 1, QCH], F32, tag="yt", bufs=2)
                            nkt = 4 * jq + 4
                            for kt in range(nkt):
                                k0 = b * T + kt * P
                                st = ps.tile([P, QCH], F32, tag="mm", bufs=3)
                                nc.tensor.matmul(
                                    st[:],
                                    kT_sb[rsl, k0:k0 + P],
                                    qT_sb[rsl, q0:q0 + QCH],
                                    start=True,
                                    stop=True,
                                )
                                pt = sb.tile([P, QCH], BF, tag="pt", bufs=4)
                                nc.scalar.activation(
                                    pt[:], st[:], AF.Exp, scale=float(SCALE)
                                )
                                i = kt - 4 * jq
                                if i >= 0:
                                    # keep where q >= k:
                                    # (-1)*p + 1*y + (-128*i) >= 0
                                    nc.gpsimd.affine_select(
                                        out=pt[:],
                                        in_=pt[:],
                                        compare_op=mybir.AluOpType.is_ge,
                                        fill=0.0,
                                        base=-P * i,
                                        channel_multiplier=-1,
                                        pattern=[[1, QCH]],
                                    )
                                nc.tensor.matmul(
                                    yt[:],
                                    v_sb[:, b * NKT + kt,
                                         h * (D + 1):(h + 1) * (D + 1)],
                                    pt[:],
                                    start=(kt == 0),
                                    stop=(kt == nkt - 1),
                                )
                            # normalize: yT[0:64] / yT[64]
                            rec = sb.tile([1, QCH], F32, tag="rec", bufs=2)
                            nc.vector.reciprocal(rec[:], yt[D:D + 1, :])
                            bc = sb.tile([D, QCH], F32, tag="bc", bufs=2)
                            nc.gpsimd.partition_broadcast(bc[:], rec[:])
                            nc.vector.tensor_mul(
                                yT_all[rsl, q0:q0 + QCH], yt[0:D, :], bc[:]
                            )

                # ---- phase 3: AllGather yT shards -> yT_full ----
                ag_in = dram.tile([DL, TT], BF, name="ag_in")
                ytf = dram.tile([C, TT], BF, name="ytf", addr_space="Shared")
                nc.sync.dma_start(ag_in[:], yT_all[:])
                nc.gpsimd.collective_compute(
                    "AllGather",
                    mybir.AluOpType.bypass,
                    replica_groups=[list(range(N_CORES))],
                    ins=[ag_in[:]],
                    outs=[ytf[:]],
                )

            # ---- phase 4: output projection (xT pool released) ----
            with tc.tile_pool(name="p4", bufs=1) as p4:
                yf_sb = p4.tile([P, NCH, TT], BF, name="yf_sb")
                for ci in range(NCH):
                    nc.sync.dma_start(
                        yf_sb[:, ci, :], ytf[ci * P:(ci + 1) * P, :]
                    )
                for tch in range(TT // QCH):
                    tsl = slice(tch * QCH, (tch + 1) * QCH)
                    po = ps.tile([P, QCH], F32, tag="mm", bufs=3)
                    for ci in range(NCH):
                        nc.tensor.matmul(
                            po[:],
                            wo_sb[:, ci, :],
                            yf_sb[:, ci, tsl],
                            start=(ci == 0),
                            stop=(ci == NCH - 1),
                        )
                    ob = p4.tile([P, QCH], F32, tag="ob", bufs=2)
                    _scopy(nc, ob[:], po[:])
                    nc.sync.dma_start(out[:, tsl], ob[:])

    nc.finalize()
    return nc


_GRAPH = None


def _get_graph():
    global _GRAPH
    if _GRAPH is None:
        _GRAPH = build_graph()
    return _GRAPH


def prepare_in_maps(x, Wq, Wk, Wv, Wo):
    x = np.asarray(x, np.float32)
    Wq = np.asarray(Wq, np.float32)
    Wk = np.asarray(Wk, np.float32)
    Wv = np.asarray(Wv, np.float32)
    Wo = np.asarray(Wo, np.float32)

    bf = ml_dtypes.bfloat16
    xTh = np.ascontiguousarray(x.reshape(TT, C).T).astype(bf)
    in_maps = []
    for r in range(N_CORES):
        sl = slice(r * DL, (r + 1) * DL)
        in_maps.append({
            "xT": xTh,
            "wqT": np.ascontiguousarray(Wq[sl].T).astype(bf),
            "wkT": np.ascontiguousarray(Wk[sl].T).astype(bf),
            "wvT": np.ascontiguousarray(Wv[sl].T).astype(bf),
            "woT": np.ascontiguousarray(Wo[sl].T).astype(bf),
        })
    return in_maps


def assemble_output(results):
    outT = np.concatenate(
        [np.asarray(results[r]["out"], np.float32) for r in range(N_CORES)],
        axis=0,
    )  # [C, TT]
    return np.ascontiguousarray(outT.T).reshape(B, T, C)


def kernel(x, Wq, Wk, Wv, Wo):
    nc = _get_graph()
    in_maps = prepare_in_maps(x, Wq, Wk, Wv, Wo)
    res = run_bass_kernel_spmd(nc, in_maps, core_ids=list(range(N_CORES)))
    return assemble_output(res.results)


# revision 3
# speedup vs baseline: 1.2575x; 1.2575x over previous
"""Causal multi-head attention block (B=2, T=2048, C=1024, H=16) on 8 TRN2
NeuronCores.

Sharding: Megatron-style tensor parallel over heads. Core r owns heads
{2r, 2r+1} (output dims [128r, 128r+128) of Wq/Wk/Wv). The final output
projection contracts over all of C, so cores AllGather their local yT
shards (concat on the partition axis == feature axis) into yT_full
[C, B*T], then each core computes a 128-column shard of the output:
outT_shard = Wo[128r:128r+128, :] @ yT_full.

Everything on-device is computed in the "transposed" orientation
(feature-major, token-minor) so the TensorEngine contraction axis always
sits on SBUF partitions and the softmax denominator arrives for free via
a ones-column appended to V:

  qT/kT/vT [128, 4096] = W_shard @ x^T          (x^T passed from host)
  ST tile [128k, 512q] = kT_slice.T @ qT_slice  (contract d=64)
  PT = exp(ST * 1/sqrt(d))                      (no max-subtraction: logits
                                                 are ~N(0,1), |S|max ~ 6)
  causal mask: zero PT where k > q via gpsimd.affine_select
  yT [65, 512] += [v | 1].T @ PT                (row 64 = softmax denom)
  yT_norm = yT[0:64] / broadcast(yT[64])

k-tiles are processed in pairs sharing one 2-bank PSUM tile so each EXP
covers 1024 columns (the ACT engine has a ~352-cycle fixed cost per
instruction). The AllGather is split into 4 chunks (per batch x half) so
gather and output projection overlap the remaining attention compute.

Inputs are bf16 (host-side cast); accumulation is f32 in PSUM.
"""

import numpy as np
import ml_dtypes

import concourse.bacc as bacc
import concourse.mybir as mybir
import concourse.tile as tile
from concourse.bass_utils import run_bass_kernel_spmd
from concourse.masks import make_identity

N_CORES = 8
B, T, C, H = 2, 2048, 1024, 16
D = 64                # head dim
HL = H // N_CORES     # heads per core = 2
DL = HL * D           # local feature dim = 128
TT = B * T            # 4096 tokens total
P = 128
NCH = C // P          # 8 contraction chunks
QCH = 512             # q-chunk (moving free dim)
NQC = T // QCH        # 4 q-chunks per batch entry
NKT = T // P          # 16 k-tiles per batch entry
HCH = T // 2          # AllGather chunk = half batch-entry = 1024 tokens
SCALE = 1.0 / np.sqrt(D)

BF = mybir.dt.bfloat16
F32 = mybir.dt.float32
AF = mybir.ActivationFunctionType


def build_graph():
    nc = bacc.Bacc("TRN2", target_bir_lowering=False, debug=False)

    xT = nc.dram_tensor("xT", [C, TT], BF, kind="ExternalInput")
    wqT = nc.dram_tensor("wqT", [C, DL], BF, kind="ExternalInput")
    wkT = nc.dram_tensor("wkT", [C, DL], BF, kind="ExternalInput")
    wvT = nc.dram_tensor("wvT", [C, DL], BF, kind="ExternalInput")
    woT = nc.dram_tensor("woT", [C, DL], BF, kind="ExternalInput")
    out = nc.dram_tensor("out", [DL, TT], F32, kind="ExternalOutput")

    with tile.TileContext(nc) as tc:
        with (
            tc.tile_pool(name="sb", bufs=1) as sb,
            tc.tile_pool(name="ps", bufs=1, space="PSUM") as ps,
            tc.tile_pool(name="dram", bufs=1, space="DRAM") as dram,
        ):
            # ---- phase 0: loads ----
            wq_sb = sb.tile([P, NCH, DL], BF, name="wq_sb")
            wk_sb = sb.tile([P, NCH, DL], BF, name="wk_sb")
            wv_sb = sb.tile([P, NCH, DL], BF, name="wv_sb")
            wo_sb = sb.tile([P, NCH, DL], BF, name="wo_sb")
            for wsb, wdram in (
                (wq_sb, wqT), (wk_sb, wkT), (wv_sb, wvT), (wo_sb, woT)
            ):
                nc.sync.dma_start(
                    wsb[:], wdram[:].rearrange("(a p) m -> p a m", p=P)
                )

            ident = sb.tile([P, P], BF, name="ident")
            make_identity(nc, ident)

            qT_sb = sb.tile([P, TT], BF, name="qT_sb")
            kT_sb = sb.tile([P, TT], BF, name="kT_sb")
            vT_sb = sb.tile([P, TT], BF, name="vT_sb")
            # v in natural layout, packed per 128-token tile as
            # [headA(64) | 1 | headB(64) | 1] -> 130 columns
            v_sb = sb.tile([P, TT // P, 2 * (D + 1)], BF, name="v_sb")
            nc.gpsimd.memset(v_sb[:], 1.0)

            yT_all = sb.tile([P, TT], BF, name="yT_all")

            with tc.tile_pool(name="xp", bufs=1) as xp:
                xT_sb = xp.tile([P, NCH, TT], BF, name="xT_sb")
                # tch-major so the first projection's operands land first
                for tch in range(TT // QCH):
                    tsl = slice(tch * QCH, (tch + 1) * QCH)
                    for ci in range(NCH):
                        nc.sync.dma_start(
                            xT_sb[:, ci, tsl], xT[ci * P:(ci + 1) * P, tsl]
                        )

                # ---- phase 1: QKV projections (transposed layout) ----
                for tch in range(TT // QCH):
                    tsl = slice(tch * QCH, (tch + 1) * QCH)
                    for wsb, dst in (
                        (wq_sb, qT_sb), (wk_sb, kT_sb), (wv_sb, vT_sb)
                    ):
                        pj = ps.tile([P, QCH], F32, tag="mm", bufs=2)
                        for ci in range(NCH):
                            nc.tensor.matmul(
                                pj[:],
                                wsb[:, ci, :],
                                xT_sb[:, ci, tsl],
                                start=(ci == 0),
                                stop=(ci == NCH - 1),
                            )
                        nc.vector.tensor_copy(dst[:, tsl], pj[:])
                    # transpose this chunk of vT into v_sb (natural layout)
                    for t32 in range(tch * (QCH // P), (tch + 1) * (QCH // P)):
                        tr = ps.tile([P, P], BF, tag="mm", bufs=2)
                        nc.tensor.transpose(
                            tr[:], vT_sb[:, t32 * P:(t32 + 1) * P], ident[:]
                        )
                        out_ap = v_sb[:, t32, :].rearrange(
                            "p (h x) -> p h x", h=HL
                        )[:, :, 0:D]
                        in_ap = tr[:].rearrange("p (h x) -> p h x", h=HL)
                        nc.vector.tensor_copy(out_ap, in_ap)

                # ---- phase 2+3+4: attention, chunked AllGather, O-proj ----
                ag_in = [
                    dram.tile([DL, HCH], BF, name=f"ag_in{c}")
                    for c in range(4)
                ]
                ytf = [
                    dram.tile([C, HCH], BF, name=f"ytf{c}", addr_space="Shared")
                    for c in range(4)
                ]

                def attn_chunk(b, jq, h):
                    rsl = slice(h * D, (h + 1) * D)
                    q0 = b * T + jq * QCH
                    yt = ps.tile([D + 1, QCH], F32, tag="yt", bufs=2)
                    nkt = 4 * jq + 4
                    for pr in range(nkt // 2):
                        st = ps.tile([P, 2 * QCH], F32, tag="st", bufs=2)
                        pt = sb.tile([P, 2 * QCH], BF, tag="pt", bufs=3)
                        for half in range(2):
                            kt = 2 * pr + half
                            k0 = b * T + kt * P
                            ssl = slice(half * QCH, (half + 1) * QCH)
                            nc.tensor.matmul(
                                st[:, ssl],
                                kT_sb[rsl, k0:k0 + P],
                                qT_sb[rsl, q0:q0 + QCH],
                                start=True,
                                stop=True,
                            )
                        nc.scalar.activation(
                            pt[:], st[:], AF.Exp, scale=float(SCALE)
                        )
                        for half in range(2):
                            kt = 2 * pr + half
                            i = kt - 4 * jq
                            if i >= 0:
                                # keep where q >= k:
                                # (-1)*p + 1*y + (-128*i) >= 0
                                nc.gpsimd.affine_select(
                                    out=pt[:, half * QCH:(half + 1) * QCH],
                                    in_=pt[:, half * QCH:(half + 1) * QCH],
                                    compare_op=mybir.AluOpType.is_ge,
                                    fill=0.0,
                                    base=-P * i,
                                    channel_multiplier=-1,
                                    pattern=[[1, QCH]],
                                )
                        for half in range(2):
                            kt = 2 * pr + half
                            nc.tensor.matmul(
                                yt[:],
                                v_sb[:, b * NKT + kt,
                                     h * (D + 1):(h + 1) * (D + 1)],
                                pt[:, half * QCH:(half + 1) * QCH],
                                start=(kt == 0),
                                stop=(kt == nkt - 1),
                            )
                    # normalize: yT[0:64] * recip(yT[64]) (denominator row)
                    den = sb.tile([1, QCH], F32, tag="den", bufs=3)
                    nc.vector.tensor_copy(den[:], yt[D:D + 1, :])
                    bc = sb.tile([D, QCH], F32, tag="bc", bufs=3)
                    nc.gpsimd.partition_broadcast(bc[:], den[:])
                    rcp = sb.tile([D, QCH], F32, tag="rcp", bufs=3)
                    scr = sb.tile([D, QCH], F32, tag="scr", bufs=3)
                    nc.vector.reciprocal_approx_accurate(
                        rcp[:], bc[:], scratch=scr[:]
                    )
                    nc.vector.tensor_mul(
                        yT_all[rsl, q0:q0 + QCH], yt[0:D, :], rcp[:]
                    )

                def oproj_chunk(c):
                    # gather chunk c (tokens [c*HCH, (c+1)*HCH)) and compute
                    # the local 128-column shard of the output projection
                    csl = slice(c * HCH, (c + 1) * HCH)
                    nc.sync.dma_start(ag_in[c][:], yT_all[:, csl])
                    nc.gpsimd.collective_compute(
                        "AllGather",
                        mybir.AluOpType.bypass,
                        replica_groups=[list(range(N_CORES))],
                        ins=[ag_in[c][:]],
                        outs=[ytf[c][:]],
                    )
                    yf = sb.tile([P, NCH, HCH], BF, tag="yf", bufs=2)
                    for ci in range(NCH):
                        nc.sync.dma_start(
                            yf[:, ci, :], ytf[c][ci * P:(ci + 1) * P, :]
                        )
                    for tch in range(HCH // QCH):
                        po = ps.tile([P, QCH], F32, tag="mm", bufs=2)
                        for ci in range(NCH):
                            nc.tensor.matmul(
                                po[:],
                                wo_sb[:, ci, :],
                                yf[:, ci, tch * QCH:(tch + 1) * QCH],
                                start=(ci == 0),
                                stop=(ci == NCH - 1),
                            )
                        ob = sb.tile([P, QCH], F32, tag="ob", bufs=2)
                        nc.vector.tensor_copy(ob[:], po[:])
                        nc.sync.dma_start(
                            out[:, c * HCH + tch * QCH:
                                c * HCH + (tch + 1) * QCH],
                            ob[:],
                        )

                for b in range(B):
                    for jq in range(NQC):
                        for h in range(HL):
                            attn_chunk(b, jq, h)
                        if jq % 2 == 1:
                            oproj_chunk(b * 2 + jq // 2)

    nc.finalize()
    return nc


_GRAPH = None


def _get_graph():
    global _GRAPH
    if _GRAPH is None:
        _GRAPH = build_graph()
    return _GRAPH


def prepare_in_maps(x, Wq, Wk, Wv, Wo):
    x = np.asarray(x, np.float32)
    Wq = np.asarray(Wq, np.float32)
    Wk = np.asarray(Wk, np.float32)
    Wv = np.asarray(Wv, np.float32)
    Wo = np.asarray(Wo, np.float32)

    bf = ml_dtypes.bfloat16
    xTh = np.ascontiguousarray(x.reshape(TT, C).T).astype(bf)
    in_maps = []
    for r in range(N_CORES):
        sl = slice(r * DL, (r + 1) * DL)
        in_maps.append({
            "xT": xTh,
            "wqT": np.ascontiguousarray(Wq[sl].T).astype(bf),
            "wkT": np.ascontiguousarray(Wk[sl].T).astype(bf),
            "wvT": np.ascontiguousarray(Wv[sl].T).astype(bf),
            "woT": np.ascontiguousarray(Wo[sl].T).astype(bf),
        })
    return in_maps


def assemble_output(results):
    outT = np.concatenate(
        [np.asarray(results[r]["out"], np.float32) for r in range(N_CORES)],
        axis=0,
    )  # [C, TT]
    return np.ascontiguousarray(outT.T).reshape(B, T, C)


def kernel(x, Wq, Wk, Wv, Wo):
    nc = _get_graph()
    in_maps = prepare_in_maps(x, Wq, Wk, Wv, Wo)
    res = run_bass_kernel_spmd(nc, in_maps, core_ids=list(range(N_CORES)))
    return assemble_output(res.results)
